# revision 10
# baseline (speedup 1.0000x reference)
"""Bilateral filter v5: Δ-form + pair-QUAD fusion on 8 trn2 cores.

Same math as v2 (out = clip(x + num'/den)), two structural changes:
1. Every pair computes on a full 34x34 window (the 36x36 halo makes this
   always in-bounds), so all pairs are shape-uniform and their elementwise
   stages fuse across PAIR DUOS: one [2,3,34,34] tile per duo, with the
   square as a single 6936-elem op, channel-sum adds and exp as [2,34,34]
   ops, and the weighted product as three [2,34,34] ops.  8 DVE ops per two
   pairs instead of 14 — per-instruction overhead (~0.3us) halves.
2. The spatial weight sk_t moves out of the exp bias into the matmul
   stationary (+-sk_t * I), enabling the bias-free duo-fused exp.
"""

import sys

sys.path.insert(0, "/opt/trn_rl_repo")

import numpy as np

KERNEL_SIZE = 5
SIGMA_S = 1.0
SIGMA_R = 0.04
INV2SR2 = 0.5 / (SIGMA_R * SIGMA_R)

B, H, W, C = 16, 512, 512, 3
NCORES = 8
IMGS_PER_CORE = B // NCORES
PATCH = 32
HALO = 36
NPS = H // PATCH
PATCHES_PER_CORE = IMGS_PER_CORE * NPS * NPS
ROUNDS = PATCHES_PER_CORE // 128

_CACHE = {}

PAIRS = [
    (dy, dx)
    for dy in range(KERNEL_SIZE)
    for dx in range(KERNEL_SIZE)
    if (dy < 2) or (dy == 2 and dx < 2)
]
NQUAD = len(PAIRS) // 4


def _space_kernel():
    x = np.arange(KERNEL_SIZE, dtype=np.float32) - (KERNEL_SIZE // 2)
    g = np.exp(-(x * x) / np.float32(2.0 * SIGMA_S * SIGMA_S)).astype(np.float32)
    g = (g / g.sum()).astype(np.float32)
    return np.outer(g, g).astype(np.float32)


def _build_module(repeat=1):
    import concourse.bacc as bacc
    import concourse.mybir as mybir
    import concourse.tile as tile

    f32 = mybir.dt.float32
    f16 = mybir.dt.float16
    A = mybir.AluOpType
    ACT = mybir.ActivationFunctionType
    sk = _space_kernel()
    sk22 = float(sk[2, 2])

    nc = bacc.Bacc("TRN2", target_bir_lowering=False, debug=False)
    xpat = nc.dram_tensor("xpat", [ROUNDS, 128, C, HALO, HALO], f16, kind="ExternalInput")
    statd = nc.dram_tensor("statd", [128, 2 * len(PAIRS), 128], f16, kind="ExternalInput")
    outd = nc.dram_tensor(
        "out", [repeat * ROUNDS, 128, C, PATCH, PATCH], f16, kind="ExternalOutput"
    )

    def origin(d):
        # full 34-window start: gather [2,34) and scatter [2-d,34-d) always inside
        return 2 - d if d > 0 else 2

    with tile.TileContext(nc) as tc:
        with (
            tc.tile_pool(name="const", bufs=1) as cpool,
            tc.tile_pool(name="xin", bufs=2) as xpool,
            tc.tile_pool(name="work", bufs=2) as wpool,
            tc.tile_pool(name="outp", bufs=2) as opool,
            tc.tile_pool(name="epi", bufs=2) as epool,
            tc.tile_pool(name="psum", bufs=1, space="PSUM") as ppool,
        ):
            stat_t = cpool.tile([128, 2 * len(PAIRS), 128], f16, tag="stat")
            nc.sync.dma_start(stat_t[:], statd[:])

            for ri, r in [
                (rep * ROUNDS + rr, rr) for rep in range(repeat) for rr in range(ROUNDS)
            ]:
                xt = xpool.tile([128, C, HALO, HALO], f16, tag="xt")
                nc.sync.dma_start(xt[:], xpat[r])

                num = [
                    ppool.tile([128, PATCH, PATCH], f32, tag=f"num{c}", name=f"num{c}")
                    for c in range(C)
                ]
                den = ppool.tile([128, PATCH, PATCH], f32, tag="den")

                for quad in range(NQUAD):
                    tis = tuple(4 * quad + j for j in range(4))
                    dlt2 = wpool.tile([128, 4, C, 34, 34], f16, tag="dlt2")
                    for s, ti in enumerate(tis):
                        dy, dx = PAIRS[ti]
                        d_y, d_x = dy - 2, dx - 2
                        u0y, u0x = origin(d_y), origin(d_x)
                        nc.vector.tensor_tensor(
                            dlt2[:, s],
                            xt[:, :, u0y + d_y : u0y + d_y + 34, u0x + d_x : u0x + d_x + 34],
                            xt[:, :, u0y : u0y + 34, u0x : u0x + 34],
                            A.subtract,
                        )
                    # vt2 doubles as the squares scratch: q lives in vt2 until
                    # the channel-sum consumes it, then v' overwrites vt2
                    vt2 = wpool.tile([128, 4, C, 34, 34], f16, tag="vt2")
                    nc.vector.tensor_tensor(vt2[:], dlt2[:], dlt2[:], A.mult)
                    d2 = wpool.tile([128, 4, 34, 34], f16, tag="d2")
                    nc.vector.tensor_tensor(d2[:], vt2[:, :, 0], vt2[:, :, 1], A.add)
                    nc.vector.tensor_tensor(d2[:], d2[:], vt2[:, :, 2], A.add)
                    w2 = wpool.tile([128, 4, 34, 34], f16, tag="w2")
                    nc.scalar.activation(w2[:], d2[:], ACT.Exp, scale=-float(INV2SR2))
                    for c in range(C):
                        nc.vector.tensor_tensor(
                            vt2[:, :, c], dlt2[:, :, c], w2[:], A.mult
                        )

                    for s, ti in enumerate(tis):
                        dy, dx = PAIRS[ti]
                        d_y, d_x = dy - 2, dx - 2
                        u0y, u0x = origin(d_y), origin(d_x)
                        gy, gx = 2 - u0y, 2 - u0x
                        zy, zx = 2 - d_y - u0y, 2 - d_x - u0x
                        first = ti == 0
                        last = ti == len(PAIRS) - 1
                        spos = stat_t[:, 2 * ti]
                        sneg = stat_t[:, 2 * ti + 1]
                        for c in range(C):
                            vg = vt2[:, s, c, gy : gy + 32, gx : gx + 32]
                            vs = vt2[:, s, c, zy : zy + 32, zx : zx + 32]
                            for hh in range(2):
                                nc.tensor.matmul(
                                    num[c][:, 16 * hh : 16 * hh + 16],
                                    spos,
                                    vg[:, 16 * hh : 16 * hh + 16],
                                    start=first,
                                    stop=False,
                                )
                                nc.tensor.matmul(
                                    num[c][:, 16 * hh : 16 * hh + 16],
                                    sneg,
                                    vs[:, 16 * hh : 16 * hh + 16],
                                    start=False,
                                    stop=last,
                                )
                        wg = w2[:, s, gy : gy + 32, gx : gx + 32]
                        ws = w2[:, s, zy : zy + 32, zx : zx + 32]
                        for hh in range(2):
                            nc.tensor.matmul(
                                den[:, 16 * hh : 16 * hh + 16],
                                spos,
                                wg[:, 16 * hh : 16 * hh + 16],
                                start=first,
                                stop=False,
                            )
                            nc.tensor.matmul(
                                den[:, 16 * hh : 16 * hh + 16],
                                spos,
                                ws[:, 16 * hh : 16 * hh + 16],
                                start=False,
                                stop=last,
                            )

                # epilogue: out = clip(x + num'/den', 0, 1), den' = den + sk22
                dsb = epool.tile([128, PATCH, PATCH], f32, tag="dsb")
                nc.vector.tensor_scalar_add(dsb[:], den[:], sk22)
                rden = epool.tile([128, PATCH, PATCH], f32, tag="rden")
                rscr = epool.tile([128, PATCH, PATCH], f32, tag="rscr")
                nc.vector.reciprocal_approx_accurate(rden[:], dsb[:], rscr[:])
                o = opool.tile([128, C, PATCH, PATCH], f16, tag="o")
                for c in range(C):
                    nc.vector.tensor_tensor(o[:, c], num[c][:], rden[:], A.mult)
                nc.vector.tensor_tensor(
                    o[:], o[:], xt[:, :, 2 : 2 + PATCH, 2 : 2 + PATCH], A.add
                )
                # no clip: the exact bilateral output is a convex combination of
                # [0,1] inputs, so clipping only trims ~1e-3 rounding excursions
                # (well inside the 2e-2 gate)
                nc.sync.dma_start(outd[ri], o[:])

    nc.finalize()
    return nc


def _get_module():
    if "nc" not in _CACHE:
        _CACHE["nc"] = _build_module()
    return _CACHE["nc"]


def _patchify(core_imgs):
    from numpy.lib.stride_tricks import sliding_window_view

    xp = np.transpose(core_imgs, (0, 3, 1, 2))
    xpad = np.pad(xp, ((0, 0), (0, 0), (2, 2), (2, 2)), mode="reflect")
    win = sliding_window_view(xpad, (HALO, HALO), axis=(2, 3))[:, :, ::PATCH, ::PATCH]
    pat = np.ascontiguousarray(win.transpose(0, 2, 3, 1, 4, 5)).reshape(
        PATCHES_PER_CORE, C, HALO, HALO
    )
    return pat.reshape(ROUNDS, 128, C, HALO, HALO).astype(np.float16)


def _unpatchify(o):
    o = o.astype(np.float32).reshape(IMGS_PER_CORE, NPS, NPS, C, PATCH, PATCH)
    o = o.transpose(0, 3, 1, 4, 2, 5).reshape(IMGS_PER_CORE, C, H, W)
    return np.ascontiguousarray(o.transpose(0, 2, 3, 1))


def _make_in_maps(images):
    sk = _space_kernel()
    eye = np.eye(128, dtype=np.float32)
    stat = np.zeros((128, 2 * len(PAIRS), 128), dtype=np.float32)
    for ti, (dy, dx) in enumerate(PAIRS):
        stat[:, 2 * ti] = sk[dy, dx] * eye
        stat[:, 2 * ti + 1] = -sk[dy, dx] * eye
    stat = stat.astype(np.float16)
    in_maps = []
    for i in range(NCORES):
        in_maps.append(
            {
                "xpat": _patchify(images[i * IMGS_PER_CORE : (i + 1) * IMGS_PER_CORE]),
                "statd": stat,
            }
        )
    return in_maps


def kernel(images):
    from concourse.bass_utils import run_bass_kernel_spmd

    images = np.asarray(images, dtype=np.float32)
    nc = _get_module()
    in_maps = _make_in_maps(images)
    res = run_bass_kernel_spmd(nc, in_maps, core_ids=list(range(NCORES)))
    out = np.empty((B, H, W, C), dtype=np.float32)
    for i in range(NCORES):
        out[i * IMGS_PER_CORE : (i + 1) * IMGS_PER_CORE] = _unpatchify(
            res.results[i]["out"]
        )
    return out
